# revision 16
# baseline (speedup 1.0000x reference)
"""Trainium2 Bass kernel for y = x @ W^T + b  (4096x4096 @ 4096x4096 + 4096).

Sharding: 2D (2 batch halves x 4 feature quarters). Core c = (bh, oq) gets
x^T[:, bh*2048:(bh+1)*2048] and W^T[:, oq*1024:(oq+1)*1024], marshalled on
the host into the device compute format (transposed layout, bf16 — the
same bf16 the kernel computes in), and produces the natural-layout f32
chunk y[bh, oq] = x_bh @ W_oq^T + b_oq. Host reassembles the 2x4 grid.

Per-core kernel (bf16 matmuls, fp32 accumulate in PSUM), engine plan:
  - x^T resident [128, 32, 2048] (128 KiB/partition), streamed as 32
    kt-chunks (4KB descriptor rows) on the gpsimd SWDGE queue.
  - W^T resident [128, 32, 1024], streamed as 4-kt superchunk halves on
    the two HWDGE queues (sync: cols 0:512, scalar: cols 512:1024) —
    transfers sized so fixed per-DMA costs don't pace the PE.
  - Warmup: a dozen dummy matmuls on memset tiles ramp the PE p-state
    to 2.4 GHz during the ~12us initial DMA latency.
  - Phase 1: 4 b-tiles x 2 psum banks accumulate kt-interleaved with the
    streams; phase 2: 12 more b-tiles with everything resident (no DMA
    dependencies at all).
  - Eviction: DVE tensor_tensor bias add per 512-span; out halves on the
    (by then idle) gpsimd queue.
"""

import os
import sys

for _p in ("/opt/trn_rl_repo", "/opt/pypackages"):
    if _p not in sys.path and os.path.isdir(_p):
        sys.path.append(_p)

import ml_dtypes
import numpy as np

import concourse.bass as bass
import concourse.tile as tile
from concourse import bacc, mybir
from concourse.bass_utils import run_bass_kernel_spmd

N_CORES = 8
BATCH = 4096
IN_F = 4096
OUT_F = 4096
P = 128
BH = 2                       # batch groups
OQ = 4                       # out-feature groups
B = BATCH // BH              # 2048 batch rows per core
O = OUT_F // OQ              # 1024 out features per core
KT = IN_F // P               # 32 contraction tiles
BT = B // P                  # 16 batch tiles per core
OS = O // 512                # 2 psum spans of 512
XW = 512                     # x block width
NXB = B // XW                # 4 x blocks
PH1 = 4                      # b-tiles in phase 1 (8 psum banks)
WGRP = 4                     # kt per W superchunk

_F32 = mybir.dt.float32
_BF16 = mybir.dt.bfloat16
_NP_BF16 = ml_dtypes.bfloat16

_compiled_nc = None


def _build():
    nc = bacc.Bacc("TRN2", target_bir_lowering=False, debug=False,
                   num_devices=N_CORES)

    xt = nc.dram_tensor("xt", [NXB, P, KT, XW], _BF16, kind="ExternalInput")
    wt = nc.dram_tensor("wt", [P, KT, O], _BF16, kind="ExternalInput")
    bias = nc.dram_tensor("bias", [P, O], _F32, kind="ExternalInput")
    out = nc.dram_tensor("out", [B, O], _F32, kind="ExternalOutput")

    with tile.TileContext(nc) as tc:
        with tc.tile_pool(name="const", bufs=1) as const, \
             tc.tile_pool(name="psum", bufs=8, space="PSUM") as psum_pool, \
             tc.tile_pool(name="yout", bufs=2) as y_pool:

            # PE p-state warmup: dummy matmuls with no DMA dependencies.
            warm = const.tile([P, 256], _BF16)
            nc.vector.memset(warm[:], 0.0)
            wps = psum_pool.tile([P, 512], _F32, name="warmps", tag="ps")
            NWARM = 44
            for i in range(NWARM):
                nc.tensor.matmul(wps[:, 0:256], lhsT=warm[:, 0:P], rhs=warm[:],
                                 start=(i == 0), stop=(i == NWARM - 1))

            bias_sb = const.tile([P, O], _F32)
            nc.scalar.dma_start(out=bias_sb[:], in_=bias[:, :])

            xsb = const.tile([P, KT, B], _BF16)
            wsb = const.tile([P, KT, O], _BF16)

            # W^T full-width kt-range superchunks (8KB contiguous
            # rows per partition), alternating across both HWDGE queues.
            wgrps = [(0, 1), (1, 2)] + [(k, min(k + 2, KT))
                                              for k in range(2, KT, 2)]
            for g, (k0, k1) in enumerate(wgrps):
                eng = nc.sync if g % 2 == 0 else nc.scalar
                eng.dma_start(out=wsb[:, k0:k1, :], in_=wt[:, k0:k1, :])

            # x block 0 in 2-kt chunks for phase-1 pacing; blocks 1..3
            # as whole contiguous DMAs that land ahead of their passes.
            xgrps = [(0, 1)] + [(k, min(k + 2, KT))
                                for k in range(1, KT, 2)]
            for (k0, k1) in xgrps:
                nc.gpsimd.dma_start(out=xsb[:, k0:k1, 0:XW],
                                    in_=xt[0, :, k0:k1, :])
            for i in range(1, NXB):
                nc.gpsimd.dma_start(out=xsb[:, :, i * XW:(i + 1) * XW],
                                    in_=xt[i, :, :, :])

            def evict(bt, ps):
                ysb = y_pool.tile([P, O], _F32, name=f"y{bt}", tag="y")
                for osp in range(OS):
                    nc.vector.tensor_tensor(
                        ysb[:, osp * 512:(osp + 1) * 512],
                        ps[osp][:],
                        bias_sb[:, osp * 512:(osp + 1) * 512],
                        mybir.AluOpType.add)
                    eng = nc.sync if osp == 0 else nc.scalar
                    eng.dma_start(
                        out=out[bt * P:(bt + 1) * P,
                                osp * 512:(osp + 1) * 512],
                        in_=ysb[:, osp * 512:(osp + 1) * 512])

            def mm(ps, bt, kt, osp, start, stop):
                nc.tensor.matmul(
                    ps[:],
                    lhsT=xsb[:, kt, bt * P:(bt + 1) * P],
                    rhs=wsb[:, kt, osp * 512:(osp + 1) * 512],
                    start=start, stop=stop)

            # ---- phase 1: first 4 b-tiles, kt-interleaved with streams
            ps1 = [[psum_pool.tile([P, 512], _F32, name=f"ps1_{bi}_{osp}",
                                   tag="ps") for osp in range(OS)]
                   for bi in range(PH1)]
            for kt in range(KT):
                for bi in range(PH1):
                    for osp in range(OS):
                        mm(ps1[bi][osp], bi, kt, osp, kt == 0, kt == KT - 1)
            for bi in range(PH1):
                evict(bi, ps1[bi])

            # ---- phase 2: remaining b-tiles, fully resident
            for bt in range(PH1, BT):
                ps = [psum_pool.tile([P, 512], _F32,
                                     name=f"ps2_{bt}_{osp}", tag="ps")
                      for osp in range(OS)]
                for kt in range(KT):
                    for osp in range(OS):
                        mm(ps[osp], bt, kt, osp, kt == 0, kt == KT - 1)
                evict(bt, ps)

    nc.compile()
    return nc


def _get_nc():
    global _compiled_nc
    if _compiled_nc is None:
        _compiled_nc = _build()
    return _compiled_nc


def _run(inputs, trace=False, trace_cores=None):
    x = np.asarray(inputs["x"], dtype=np.float32)
    w = np.asarray(inputs["weight"], dtype=np.float32)
    b = np.asarray(inputs["bias"], dtype=np.float32)

    nc = _get_nc()
    in_maps = []
    for c in range(N_CORES):
        bh, oq = divmod(c, OQ)
        xt_c = np.ascontiguousarray(
            x[bh * B:(bh + 1) * B, :].T.astype(_NP_BF16)
            .reshape(KT, P, NXB, XW).transpose(2, 1, 0, 3))
        wt_c = np.ascontiguousarray(
            w[oq * O:(oq + 1) * O, :].T.astype(_NP_BF16)
            .reshape(KT, P, O).transpose(1, 0, 2))
        bias_c = np.ascontiguousarray(
            np.broadcast_to(b[oq * O:(oq + 1) * O], (P, O)))
        in_maps.append({"xt": xt_c, "wt": wt_c, "bias": bias_c})

    res = run_bass_kernel_spmd(nc, in_maps, core_ids=list(range(N_CORES)),
                               trace=trace, trace_cores=trace_cores)
    y = np.empty((BATCH, OUT_F), dtype=np.float32)
    for c in range(N_CORES):
        bh, oq = divmod(c, OQ)
        y[bh * B:(bh + 1) * B, oq * O:(oq + 1) * O] = res.results[c]["out"]
    return y, res


def kernel(**inputs):
    y, _ = _run(inputs)
    return y
